# revision 66
# baseline (speedup 1.0000x reference)
"""Trainium2 Bass kernel: causal multi-head self-attention with RoPE.

Problem: x[2,2048,1024], 16 heads, d_k=64, causal, RoPE(theta=1e4),
out = (softmax(rope(Q)rope(K)^T/8) V) WO^T.

Sharding (8 cores): data-parallel over batch (2) x head-parallel over
head groups (4 heads per core).  Each core computes Q/K/V projections
for its 4 heads, flash-style causal attention, and a partial output
projection over its 256 channels; the host sums the 4 partials per
batch element.

Device layouts (per core, all bf16 except PSUM):
  xt  [1024,2048]  x[b]^T (d_model on partitions), shipped pre-chunked
      in the exact SBUF layout so every DMA is contiguous.
  Qt/Kt tiles [128,2048]: 2 heads each, per head rows = [32 even-dim,
      32 odd-dim] (host permutes W_Q/W_K columns) so RoPE is pure
      row-block ops; scores are permutation-invariant.
  V   [128,16,4,65]: natural [s,d] layout per 128-row s-block, 65th
      column of ones => P@[V|1] yields softmax denominators for free.
  scores computed transposed ([keys,queries]) so P^T feeds P@V with the
      contraction (keys) on partitions.  Causal masking: an identity
      matmul preloads -1e5 into the key>query region of the scores psum
      (keeps the mask off the Vector engine / out of the exp->PV chain);
      PV skips fully-masked leading columns of diagonal key blocks.
  softmax normalization is deferred: unnormalized head outputs plus the
      denominator rows are staged, then one fast-reciprocal + an
      indicator-matmul broadcast rescales everything at the tail,
      pipelined per query-slice with the output projection.
"""

import os
import sys

for _p in ("/opt/trn_rl_repo",):
    if _p not in sys.path:
        sys.path.insert(0, _p)

import numpy as np
import ml_dtypes

BF16 = ml_dtypes.bfloat16

D = 1024
S = 2048
H = 16
DK = 64
HPC = 4          # heads per core
NCORES = 8
THETA = 10000.0

_COMPILED = {}


def _build_nc():
    import concourse.bass as bass  # noqa: F401
    import concourse.bacc as bacc
    import concourse.mybir as mybir
    import concourse.tile as tile

    bf16 = mybir.dt.bfloat16
    f32 = mybir.dt.float32
    Exp = mybir.ActivationFunctionType.Exp

    nc = bacc.Bacc(
        "TRN2", target_bir_lowering=False, debug=False, num_devices=NCORES
    )
    xt_d = nc.declare_dram_parameter("xt", [4, 128, 8, 512], bf16, isOutput=False)
    wq_d = nc.declare_dram_parameter("wq", [128, 8, 256], bf16, isOutput=False)
    wk_d = nc.declare_dram_parameter("wk", [128, 8, 256], bf16, isOutput=False)
    wv_d = nc.declare_dram_parameter("wv", [128, 8, 256], bf16, isOutput=False)
    wo_d = nc.declare_dram_parameter("wo", [128, 2, D], bf16, isOutput=False)
    cos_d = nc.declare_dram_parameter("cosb", [128, S], bf16, isOutput=False)
    sin_d = nc.declare_dram_parameter("sinb", [128, S], bf16, isOutput=False)
    msk_d = nc.declare_dram_parameter("msk", [128, 4, 512], bf16, isOutput=False)
    eye_d = nc.declare_dram_parameter("eye", [128, 128], bf16, isOutput=False)
    ind_d = nc.declare_dram_parameter("ind", [40, 4, 128], bf16, isOutput=False)
    out_d = nc.declare_dram_parameter("out", [S, D], bf16, isOutput=True)

    with tile.TileContext(nc) as tc:
        with tc.tile_pool(name="const", bufs=1) as const:
            x_sb = const.tile([128, 8, S], bf16)
            wq_sb = const.tile([128, 8, 256], bf16)
            wk_sb = const.tile([128, 8, 256], bf16)
            wv_sb = const.tile([128, 8, 256], bf16)
            wo_sb = const.tile([128, 2, D], bf16)
            cos_sb = const.tile([128, S], bf16)
            sin_sb = const.tile([128, S], bf16)
            msk_sb = const.tile([128, 4, 512], bf16)
            eye_sb = const.tile([128, 128], bf16)
            ind_sb = const.tile([40, 4, 128], bf16)
            v_sb = const.tile([128, 16, 4, 65], bf16)
            qraw = [const.tile([128, S], bf16, name=f"qraw{i}") for i in range(2)]
            kraw = [const.tile([128, S], bf16, name=f"kraw{i}") for i in range(2)]
            qrot = [const.tile([128, S], bf16, name=f"qrot{i}") for i in range(2)]
            krot = [const.tile([128, S], bf16, name=f"krot{i}") for i in range(2)]
            at = [const.tile([128, S], bf16, name=f"at{i}") for i in range(2)]

            # x slices stream on the HW-DGE (sync) queue; everything else is
            # issued in parallel from the gpsimd queue
            for nsl in range(4):
                nc.sync.dma_start(
                    x_sb[:, :, nsl * 512:(nsl + 1) * 512], xt_d[nsl]
                )
            nc.gpsimd.dma_start(wq_sb[:], wq_d[:])
            nc.gpsimd.dma_start(wk_sb[:], wk_d[:])
            nc.gpsimd.dma_start(wv_sb[:], wv_d[:])
            nc.gpsimd.dma_start(cos_sb[:], cos_d[:])
            nc.gpsimd.dma_start(sin_sb[:], sin_d[:])
            nc.gpsimd.dma_start(msk_sb[:], msk_d[:])
            nc.gpsimd.dma_start(eye_sb[:], eye_d[:])
            nc.gpsimd.dma_start(ind_sb[:], ind_d[:])
            nc.gpsimd.dma_start(wo_sb[:], wo_d[:])
            nc.vector.memset(v_sb[:, :, :, 64:65], 1.0)
            # load the Exp activation table before the first real exp needs it
            warm = const.tile([1, 16], f32)
            warmo = const.tile([1, 16], bf16)
            nc.vector.memset(warm[:], 0.0)
            nc.scalar.activation(warmo[:], warm[:], Exp)

            # ---- phase 1: Q/K/V projections + RoPE ----
            with tc.tile_pool(name="pj", bufs=4, space="PSUM") as pjp, \
                 tc.tile_pool(name="pvps", bufs=2, space="PSUM") as pvps:
                # all W_Q projections before the first W_K use: wk streams
                # on the gpsimd queue ~5us behind wq, and the wq work for
                # all four slices (~14us) covers that gap (x chunk nsl
                # arrives well before its slice comes up)
                for w_sb, raw in ((wq_sb, qraw), (wk_sb, kraw)):
                    for nsl in range(4):
                        for ot in range(2):
                            ps = pjp.tile([128, 512], f32, tag="pj", name="pj")
                            for c in range(8):
                                nc.tensor.matmul(
                                    ps[:],
                                    w_sb[:, c, ot * 128:(ot + 1) * 128],
                                    x_sb[:, c, nsl * 512:(nsl + 1) * 512],
                                    start=(c == 0), stop=(c == 7),
                                )
                            nc.vector.tensor_copy(
                                raw[ot][:, nsl * 512:(nsl + 1) * 512], ps[:]
                            )
                for sb in range(16):
                    ps = pvps.tile([128, 256], f32, tag="pv", name="pv")
                    for c in range(8):
                        nc.tensor.matmul(
                            ps[:],
                            x_sb[:, c, sb * 128:(sb + 1) * 128],
                            wv_sb[:, c, :],
                            start=(c == 0), stop=(c == 7),
                        )
                    nc.vector.tensor_copy(
                        v_sb[:, sb, :, 0:64],
                        ps[:].rearrange("p (h d) -> p h d", h=4),
                    )
                with tc.tile_pool(name="rope", bufs=2) as rp:
                    for raw, rot in ((qraw, qrot), (kraw, krot)):
                        for ot in range(2):
                            sw = rp.tile([128, S], bf16, tag="sw", name="sw")
                            t1 = rp.tile([128, S], bf16, tag="t1", name="t1")
                            for blk in range(4):
                                src = blk ^ 1
                                nc.sync.dma_start(
                                    sw[blk * 32:(blk + 1) * 32, :],
                                    raw[ot][src * 32:(src + 1) * 32, :],
                                )
                            nc.vector.tensor_mul(t1[:], raw[ot][:], cos_sb[:])
                            nc.vector.tensor_mul(sw[:], sw[:], sin_sb[:])
                            nc.vector.tensor_add(rot[ot][:], t1[:], sw[:])

            # ---- phase 2: causal attention (scores transposed) ----
            den_sb = const.tile([40, 512], bf16)
            rc = const.tile([40, 512], f32)
            rcb = const.tile([40, 512], bf16)
            atn = [const.tile([128, 4, 512], bf16, name=f"atn{i}")
                   for i in range(2)]
            with tc.tile_pool(name="ps_s", bufs=3, space="PSUM") as psc, \
                 tc.tile_pool(name="ps_o", bufs=2, space="PSUM") as pso, \
                 tc.tile_pool(name="pp", bufs=4) as ppool, \
                 tc.tile_pool(name="nrm", bufs=3) as nrm:
                for h in range(HPC):
                    ot, hl = divmod(h, 2)
                    qr, kr = qrot[ot], krot[ot]
                    r0 = hl * 64
                    for j in range(4):
                        nkb = 4 * (j + 1)
                        po = pso.tile([65, 512], f32, tag="po", name="po")
                        for g0 in range(0, nkb, 2):
                            G = min(2, nkb - g0)
                            sp = psc.tile([128, 1024], f32, tag="sc", name="sp")
                            pt = ppool.tile([128, 1024], bf16, tag="pt", name="pt")
                            for i in range(G):
                                kb = g0 + i
                                dg = kb - 4 * j
                                # cols < dg*128 of a diagonal block are fully
                                # masked: skip them in the preload, the score
                                # matmul, exp and PV alike (mask preload and
                                # score matmul keep identical psum ranges so
                                # the accumulation-group order is honored)
                                c0 = dg * 128 if dg > 0 else 0
                                if dg >= 0:
                                    # the mask is nonzero only in the
                                    # 128-col triangle window: preload +
                                    # score there as one same-range group,
                                    # and the unmasked remainder as a plain
                                    # standalone matmul
                                    a0 = i * 512 + c0
                                    nc.tensor.matmul(
                                        sp[:, a0:a0 + 128],
                                        eye_sb[:],
                                        msk_sb[:, dg, c0:c0 + 128],
                                        start=True, stop=False,
                                    )
                                    nc.tensor.matmul(
                                        sp[:, a0:a0 + 128],
                                        kr[r0:r0 + 64,
                                           kb * 128:(kb + 1) * 128],
                                        qr[r0:r0 + 64,
                                           j * 512 + c0:j * 512 + c0 + 128],
                                        start=False, stop=True,
                                    )
                                    if c0 + 128 < 512:
                                        nc.tensor.matmul(
                                            sp[:, a0 + 128:(i + 1) * 512],
                                            kr[r0:r0 + 64,
                                               kb * 128:(kb + 1) * 128],
                                            qr[r0:r0 + 64,
                                               j * 512 + c0 + 128:
                                               (j + 1) * 512],
                                            start=True, stop=True,
                                        )
                                else:
                                    nc.tensor.matmul(
                                        sp[:, i * 512:(i + 1) * 512],
                                        kr[r0:r0 + 64,
                                           kb * 128:(kb + 1) * 128],
                                        qr[r0:r0 + 64,
                                           j * 512:(j + 1) * 512],
                                        start=True, stop=True,
                                    )
                            if g0 >= 4 * j:
                                # diagonal group: per-block exp over exactly
                                # the freshly written psum range
                                for i in range(G):
                                    c0 = (g0 + i - 4 * j) * 128
                                    nc.scalar.activation(
                                        pt[:, i * 512 + c0:(i + 1) * 512],
                                        sp[:, i * 512 + c0:(i + 1) * 512],
                                        Exp, scale=0.125,
                                    )
                            else:
                                nc.scalar.activation(
                                    pt[:, 0:G * 512], sp[:, 0:G * 512],
                                    Exp, scale=0.125,
                                )
                            for i in range(G):
                                kb = g0 + i
                                dg = kb - 4 * j
                                # cols < dg*128 of a diagonal block are fully
                                # masked (exactly 0 after exp): PV skips them
                                c0 = dg * 128 if dg > 0 else 0
                                nc.tensor.matmul(
                                    po[:, c0:512],
                                    v_sb[:, kb, h, 0:65],
                                    pt[:, i * 512 + c0:(i + 1) * 512],
                                    start=(kb == 0), stop=(kb == nkb - 1),
                                )
                        # stage unnormalized out + denominator, release po fast
                        tm = nrm.tile([65, 512], bf16, tag="tm", name="tm")
                        nc.vector.tensor_copy(tm[:], po[:])
                        nc.sync.dma_start(
                            at[ot][r0:r0 + 64, j * 512:(j + 1) * 512], tm[0:64, :]
                        )
                        dr = ot * 32 + hl * 4 + j
                        nc.sync.dma_start(den_sb[dr:dr + 1, :], tm[64:65, :])

            # ---- tail: normalization + output projection, pipelined per jsl ----
            with tc.tile_pool(name="ps_r", bufs=2, space="PSUM") as psr, \
                 tc.tile_pool(name="ps_f", bufs=4, space="PSUM") as psf, \
                 tc.tile_pool(name="ost", bufs=4) as ost:
                denf = ost.tile([40, 512], f32, tag="denf", name="denf")
                nc.vector.tensor_copy(denf[:], den_sb[:])
                nc.vector.reciprocal_approx_fast(rc[:], denf[:])
                nc.vector.tensor_copy(rcb[:], rc[:])
                for jsl in range(4):
                    for ot in range(2):
                        rbp = psr.tile([128, 512], f32, tag="rb", name="rb")
                        nc.tensor.matmul(
                            rbp[:], ind_sb[ot * 32:ot * 32 + 8, jsl, :],
                            rcb[ot * 32:ot * 32 + 8, :], start=True, stop=True,
                        )
                        nc.vector.tensor_mul(
                            atn[ot][:, jsl, :],
                            at[ot][:, jsl * 512:(jsl + 1) * 512],
                            rbp[:],
                        )
                    for sbi in range(4):
                        sb = jsl * 4 + sbi
                        for osl in range(2):
                            pf = psf.tile([128, 512], f32, tag="pf", name="pf")
                            for ich in range(2):
                                nc.tensor.matmul(
                                    pf[:],
                                    atn[ich][:, jsl, sbi * 128:(sbi + 1) * 128],
                                    wo_sb[:, ich, osl * 512:(osl + 1) * 512],
                                    start=(ich == 0), stop=(ich == 1),
                                )
                            ob = ost.tile([128, 512], bf16, tag="ob", name="ob")
                            nc.scalar.copy(ob[:], pf[:])
                            nc.sync.dma_start(
                                out_d[sb * 128:(sb + 1) * 128,
                                      osl * 512:(osl + 1) * 512],
                                ob[:],
                            )
    nc.compile()
    return nc


def _host_prep(x, token_positions, WQ, WK, WV, WO):
    """Build the 8 per-core input maps."""
    pos = np.asarray(token_positions).astype(np.float32)
    k = np.arange(DK // 2, dtype=np.float32)
    inv_freq = 1.0 / (THETA ** (2.0 * k / DK))
    ang = pos[:, None] * inv_freq[None, :]          # [S, 32]
    c32 = np.cos(ang).T.astype(np.float32)          # [32, S]
    s32 = np.sin(ang).T.astype(np.float32)
    cosb = np.tile(c32, (4, 1)).astype(BF16)        # [128, S]
    sinb = np.concatenate([-s32, s32, -s32, s32], axis=0).astype(BF16)
    # causal masks for the 4 diagonal key-blocks of a 512-query slice
    kk = np.arange(128)[:, None, None]
    dd = np.arange(4)[None, :, None]
    qq = np.arange(512)[None, None, :]
    msk = np.where(dd * 128 + kk <= qq, 0.0, -1e5).astype(BF16)  # [128, 4, 512]
    eye = np.eye(128, dtype=np.float32).astype(BF16)
    # indicator matrices for denominator broadcast:
    # ind[i, jsl, r] = 1 iff i == (r//64)*4 + jsl  (same for both head pairs)
    ind = np.zeros((40, 4, 128), dtype=np.float32)
    for jsl in range(4):
        for r in range(128):
            ind[(r // 64) * 4 + jsl, jsl, r] = 1.0
            ind[32 + (r // 64) * 4 + jsl, jsl, r] = 1.0
    ind = ind.astype(BF16)

    perm = np.concatenate([np.arange(0, DK, 2), np.arange(1, DK, 2)])  # evens,odds

    in_maps = []
    for core in range(NCORES):
        b, hg = divmod(core, 4)
        ch0 = hg * 256
        qk_rows = np.concatenate([ch0 + hl * 64 + perm for hl in range(HPC)])
        def dev_w(w):  # [D, M] -> [128, 8, M] (contraction chunks)
            return np.ascontiguousarray(
                w.reshape(8, 128, -1).transpose(1, 0, 2)
            ).astype(BF16)

        xt = np.asarray(x[b]).T                       # [D, S]
        xt4 = np.ascontiguousarray(
            xt.reshape(8, 128, 4, 512).transpose(2, 1, 0, 3)
        ).astype(BF16)                                # [4, 128, 8, 512]
        in_maps.append({
            "xt": xt4,
            "wq": dev_w(np.asarray(WQ)[qk_rows, :].T),
            "wk": dev_w(np.asarray(WK)[qk_rows, :].T),
            "wv": dev_w(np.asarray(WV)[ch0:ch0 + 256, :].T),
            "wo": np.ascontiguousarray(
                np.asarray(WO)[:, ch0:ch0 + 256].T.reshape(2, 128, D)
                .transpose(1, 0, 2)
            ).astype(BF16),
            "cosb": cosb,
            "sinb": sinb,
            "msk": msk,
            "eye": eye,
            "ind": ind,
        })
    return in_maps


LAST_EXEC_NS = None


def kernel(x, token_positions, WQ, WK, WV, WO):
    global LAST_EXEC_NS
    from concourse.bass_utils import run_bass_kernel_spmd

    if "nc" not in _COMPILED:
        _COMPILED["nc"] = _build_nc()
    nc = _COMPILED["nc"]

    in_maps = _host_prep(x, token_positions, WQ, WK, WV, WO)
    res = run_bass_kernel_spmd(nc, in_maps, list(range(NCORES)))
    LAST_EXEC_NS = res.exec_time_ns

    out = np.zeros((2, S, D), dtype=np.float32)
    for core in range(NCORES):
        out[core // 4] += np.asarray(res.results[core]["out"], dtype=np.float32)
    return out
